# revision 11
# baseline (speedup 1.0000x reference)
"""Trainium2 Bass kernel for nn_AttentionLayer_77558519431766.

Math: the reference computes softmax over a size-1 axis, which is
identically 1.0, so the attention MLP is dead code and

    out[b, e] = sum_{i<j} x[b,i,e] * x[b,j,e]
              = 0.5 * ((sum_f x[b,f,e])^2 - sum_f x[b,f,e]^2)

Implementation v4 (PE reduction pipeline, column-sliced DMA):
  Per 128-row chunk the 50 f-planes stream in as column slices so compute
  starts ~4us in instead of waiting for the whole chunk. Per slice:
    - DVE casts f32 -> bf16 (2x_2p)
    - PE transposes each [128b, 128(f2,e)] block into PSUM
    - DVE + ACT split the PSUM->SBUF copyback (xT for the s-chain);
      ACT squares the transposed blocks into SBUF (sq, scaled by 0.5)
    - PE accumulates s = sum_f x and q = 0.5*sum_f x^2 via one-hot
      stacked masks [128,64], grouped per slice to limit LDWEIGHTS churn
  Chunk combine: res = Square(s*sqrt(.5)) - q, one small DMA out.
  ~30 dummy matmuls at t=0 warm the PE HAM clock gate (1.2 -> 2.4 GHz)
  under the first DMA's shadow.

Sharding: pure data parallelism, batch 2048 -> 8 shards of 256.
"""

import numpy as np

try:
    import concourse.bass as bass  # noqa: F401
except ImportError:  # pragma: no cover
    import sys

    sys.path.insert(0, "/opt/trn_rl_repo")

_B, _F, _E = 2048, 50, 64
_NCORES = 8
_BS = _B // _NCORES  # 256 rows per core
_ROW = _F * _E  # 3200 floats per row
_P = 128  # SBUF partitions
_NCHUNK = _BS // _P  # 2
# f-planes per DMA slice, per chunk (must be even: 1 block = 2 planes).
# Last slice of the last chunk is small to shorten the critical tail.
_SLICES = [
    [6, 14, 14, 16],
    [16, 16, 16, 2],
]
_WARM_MMS = 30  # dummy matmuls to release the PE HAM clock gate


def _make_tc_class():
    """TileContext with a slim kernel tail (drops the redundant tail
    sem-clear + second barrier; the Bass preamble re-clears at start)."""
    from concourse.tile import TileContext
    from concourse.vector_clock import ScopedClock

    class SlimTailTileContext(TileContext):
        def _drain_and_barrier(self, tick_clock, wait_clock):
            drain_inst = self.nc.sync.drain()
            wait_clock.add_sem_waits(
                drain_inst.ins, ScopedClock({None: tick_clock.global_clock})
            )
            self.nc.all_engine_barrier(sem_only=True)
            popped = self.nc._tile_sem_poison_stack.pop()
            assert popped is self._sem_poison

    return SlimTailTileContext


def _build():
    import concourse.bacc as bacc
    import concourse.mybir as mybir

    TileContext = _make_tc_class()

    f32 = mybir.dt.float32
    bf16 = mybir.dt.bfloat16
    i32 = mybir.dt.int32
    SQ = mybir.ActivationFunctionType.Square
    ALU = mybir.AluOpType
    HALF_SQRT = float(np.float32(np.sqrt(0.5)))

    nc = bacc.Bacc()
    x = nc.declare_dram_parameter("inputs", [_BS, _ROW], f32, isOutput=False)
    out = nc.declare_dram_parameter("out", [_NCHUNK, _P, _E], f32, isOutput=True)

    with TileContext(nc) as tc:
        with (
            tc.tile_pool(name="consts", bufs=1) as cpool,
            tc.tile_pool(name="x", bufs=6) as xpool,
            tc.tile_pool(name="xb", bufs=3) as xbpool,
            tc.tile_pool(name="xT", bufs=3) as xtpool,
            tc.tile_pool(name="xsq", bufs=3) as sqpool,
            tc.tile_pool(name="pt", bufs=3, space="PSUM") as ptpool,
            tc.tile_pool(name="acc", bufs=2, space="PSUM") as accpool,
            tc.tile_pool(name="warmps", bufs=1, space="PSUM") as wpool,
            tc.tile_pool(name="small", bufs=1) as spool,
        ):
            # ACT warm op: hoists the Square function-table load.
            warm = spool.tile([_P, 1], f32, tag="warm")
            nc.gpsimd.memset(warm[:], 0.0)
            nc.scalar.activation(warm[:], warm[:], SQ)

            # Constants built on-chip. iota with channel_multiplier=-1
            # gives v[p,j] = j - p; the stacked one-hot mask [128,64] has
            # ones where j - p is 0 or -64. Masks are pre-scaled so the
            # final combine needs no extra scaling: s-chain mask
            # sqrt(0.5)-one-hot -> s'^2 = 0.5 s^2; q-chain 0.5-one-hot.
            iot_i = cpool.tile([_P, _P], i32, tag="iot_i")
            iot_m = cpool.tile([_P, _E], i32, tag="iot_m")
            ident = cpool.tile([_P, _P], bf16, tag="ident")
            mask = cpool.tile([_P, _E], bf16, tag="mask")
            mask_b = cpool.tile([_P, _E], bf16, tag="mask_b")
            nc.gpsimd.iota(iot_i[:], pattern=[[1, _P]], base=0, channel_multiplier=-1)
            nc.gpsimd.iota(iot_m[:], pattern=[[1, _E]], base=0, channel_multiplier=-1)
            nc.vector.tensor_scalar(ident[:], iot_i[:], 0, None, op0=ALU.is_equal)
            nc.vector.tensor_scalar(mask[:], iot_m[:], 0, None, op0=ALU.is_equal)
            nc.vector.tensor_scalar(mask_b[:], iot_m[:], -_E, None, op0=ALU.is_equal)
            nc.vector.tensor_add(mask[:], mask[:], mask_b[:])
            maskh = cpool.tile([_P, _E], bf16, tag="maskh")
            maskq = cpool.tile([_P, _E], bf16, tag="maskq")
            nc.vector.tensor_scalar_mul(maskh[:], mask[:], HALF_SQRT)
            nc.vector.tensor_scalar_mul(maskq[:], mask[:], 0.5)

            # PE HAM warm-up: dummy matmuls on a zeroed tile, result never
            # read. Runs under the first input DMA's shadow and releases
            # the PE clock gate before the real transposes arrive.
            wmov = cpool.tile([_P, _P], bf16, tag="wmov")
            nc.gpsimd.memset(wmov[:], 0.0)
            # full-bank PSUM tile: start=True clears the whole bank, so no
            # other tile may share it
            wps = wpool.tile([_P, 512], f32, tag="wps")
            for _ in range(_WARM_MMS):
                nc.tensor.matmul(wps[:, :_P], wmov[:], ident[:], start=True, stop=True)

            for c in range(_NCHUNK):
                rows = slice(c * _P, (c + 1) * _P)
                slices = _SLICES[c]
                nblk_total = sum(n // 2 for n in slices)
                # full-bank PSUM accumulators (see wps comment)
                s_t = accpool.tile([_P, 512], f32, tag="s")
                q_t = accpool.tile([_P, 512], f32, tag="q")
                s_ps = s_t[:, :_E]
                q_ps = q_t[:, :_E]
                blk0 = 0
                col0 = 0
                for si, n in enumerate(slices):
                    w = n * _E
                    nblk = n // 2
                    xt = xpool.tile([_P, w], f32, tag=f"x{n}")
                    nc.sync.dma_start(out=xt[:], in_=x[rows, col0 : col0 + w])
                    xb = xbpool.tile([_P, w], bf16, tag=f"xb{n}")
                    nc.vector.tensor_copy(xb[:], xt[:])
                    pt = ptpool.tile([_P, 8 * _P], bf16, tag="pt")
                    for j in range(nblk):
                        nc.tensor.transpose(
                            pt[:, j * _P : (j + 1) * _P],
                            xb[:, j * _P : (j + 1) * _P],
                            ident[:],
                        )
                    # copyback split between DVE and ACT for balance
                    xT = xtpool.tile([_P, w], bf16, tag=f"xT{n}")
                    h = (nblk // 2) * _P
                    if h > 0:
                        nc.vector.tensor_copy(xT[:, :h], pt[:, :h])
                    if h < w:
                        nc.scalar.activation(xT[:, h:w], pt[:, h:w], mybir.ActivationFunctionType.Copy)
                    sq = sqpool.tile([_P, w], bf16, tag=f"sq{n}")
                    nc.scalar.activation(sq[:], pt[:, :w], SQ)
                    # grouped matmuls: one stationary load per chain per slice
                    for j in range(nblk):
                        kk = blk0 + j
                        bcols = slice(j * _P, (j + 1) * _P)
                        nc.tensor.matmul(
                            s_ps,
                            xT[:, bcols],
                            maskh[:],
                            start=(kk == 0),
                            stop=(kk == nblk_total - 1),
                        )
                    for j in range(nblk):
                        kk = blk0 + j
                        bcols = slice(j * _P, (j + 1) * _P)
                        nc.tensor.matmul(
                            q_ps,
                            sq[:, bcols],
                            maskq[:],
                            start=(kk == 0),
                            stop=(kk == nblk_total - 1),
                        )
                    blk0 += nblk
                    col0 += w

                # res = 0.5*s^2 - 0.5*q
                m2 = spool.tile([_P, _E], f32, tag=f"m2_{c}")
                res = spool.tile([_P, _E], f32, tag=f"res_{c}")
                nc.scalar.activation(m2[:], s_ps, SQ)
                nc.vector.tensor_sub(res[:], m2[:], q_ps)
                nc.sync.dma_start(out=out[c], in_=res[:])
    nc.compile()
    return nc


_WALRUS_EXTRA = ["--fast-context-switch"]


def _patch_walrus():
    """Hook to append extra walrus_driver args (e.g. --max-sem-num to cap
    the one-event-sem-op-per-semaphore zeroing postamble)."""
    from concourse import bass_utils

    if getattr(bass_utils, "_walrus_patched", False):
        return
    real_run = bass_utils.run_command

    def run2(cmd, **kw):
        if cmd and "walrus_driver" in str(cmd[0]):
            cmd = list(cmd) + _WALRUS_EXTRA
        return real_run(cmd, **kw)

    bass_utils.run_command = run2
    bass_utils._walrus_patched = True


def _run(in_maps, **kwargs):
    from concourse.bass_utils import run_bass_kernel_spmd

    _patch_walrus()
    nc = _build()
    return run_bass_kernel_spmd(nc, in_maps, core_ids=list(range(_NCORES)), **kwargs)


def _shard(inputs: np.ndarray):
    x = np.ascontiguousarray(
        np.asarray(inputs, dtype=np.float32).reshape(_B, _ROW)
    )
    return [
        {"inputs": np.ascontiguousarray(x[i * _BS : (i + 1) * _BS])}
        for i in range(_NCORES)
    ]


def kernel(
    inputs: np.ndarray,
    weight_attention: np.ndarray = None,
    weight_projection: np.ndarray = None,
    weight_bias: np.ndarray = None,
) -> np.ndarray:
    # weights are dead code (softmax over a size-1 axis == 1.0)
    res = _run(_shard(inputs))
    return np.concatenate(
        [r["out"].reshape(_BS, _E) for r in res.results], axis=0
    )


# revision 13
# speedup vs baseline: 1.1086x; 1.1086x over previous
"""Trainium2 Bass kernel for nn_AttentionLayer_77558519431766.

Math: the reference computes softmax over a size-1 axis, which is
identically 1.0, so the attention MLP is dead code and

    out[b, e] = sum_{i<j} x[b,i,e] * x[b,j,e]
              = 0.5 * ((sum_f x[b,f,e])^2 - sum_f x[b,f,e]^2)

Implementation v4 (PE reduction pipeline, column-sliced DMA):
  Per 128-row chunk the 50 f-planes stream in as column slices so compute
  starts ~4us in instead of waiting for the whole chunk. Per slice:
    - DVE casts f32 -> bf16 (2x_2p)
    - PE transposes each [128b, 128(f2,e)] block into PSUM
    - DVE + ACT split the PSUM->SBUF copyback (xT for the s-chain);
      ACT squares the transposed blocks into SBUF (sq, scaled by 0.5)
    - PE accumulates s = sum_f x and q = 0.5*sum_f x^2 via one-hot
      stacked masks [128,64], grouped per slice to limit LDWEIGHTS churn
  Chunk combine: res = Square(s*sqrt(.5)) - q, one small DMA out.
  ~30 dummy matmuls at t=0 warm the PE HAM clock gate (1.2 -> 2.4 GHz)
  under the first DMA's shadow.

Sharding: pure data parallelism, batch 2048 -> 8 shards of 256.
"""

import numpy as np

try:
    import concourse.bass as bass  # noqa: F401
except ImportError:  # pragma: no cover
    import sys

    sys.path.insert(0, "/opt/trn_rl_repo")

_B, _F, _E = 2048, 50, 64
_NCORES = 8
_BS = _B // _NCORES  # 256 rows per core
_ROW = _F * _E  # 3200 floats per row
_P = 128  # SBUF partitions
_NCHUNK = _BS // _P  # 2
# f-planes per DMA slice, per chunk (must be even: 1 block = 2 planes).
# Last slice of the last chunk is small to shorten the critical tail.
_SLICES = [
    [8, 16, 16, 10],
    [16, 16, 16, 2],
]
_WARM_MMS = 30  # dummy matmuls to release the PE HAM clock gate


def _make_tc_class():
    """TileContext with a slim kernel tail (drops the redundant tail
    sem-clear + second barrier; the Bass preamble re-clears at start)."""
    from concourse.tile import TileContext
    from concourse.vector_clock import ScopedClock

    class SlimTailTileContext(TileContext):
        def _drain_and_barrier(self, tick_clock, wait_clock):
            drain_inst = self.nc.sync.drain()
            wait_clock.add_sem_waits(
                drain_inst.ins, ScopedClock({None: tick_clock.global_clock})
            )
            self.nc.all_engine_barrier(sem_only=True)
            popped = self.nc._tile_sem_poison_stack.pop()
            assert popped is self._sem_poison

    return SlimTailTileContext


def _build():
    import concourse.bacc as bacc
    import concourse.mybir as mybir

    TileContext = _make_tc_class()

    f32 = mybir.dt.float32
    bf16 = mybir.dt.bfloat16
    i32 = mybir.dt.int32
    SQ = mybir.ActivationFunctionType.Square
    ALU = mybir.AluOpType
    HALF_SQRT = float(np.float32(np.sqrt(0.5)))

    nc = bacc.Bacc()
    x = nc.declare_dram_parameter("inputs", [_BS, _ROW], f32, isOutput=False)
    out = nc.declare_dram_parameter("out", [_NCHUNK, _P, _E], f32, isOutput=True)

    with TileContext(nc) as tc:
        with (
            tc.tile_pool(name="consts", bufs=1) as cpool,
            tc.tile_pool(name="x", bufs=6) as xpool,
            tc.tile_pool(name="xb", bufs=3) as xbpool,
            tc.tile_pool(name="xT", bufs=3) as xtpool,
            tc.tile_pool(name="xsq", bufs=3) as sqpool,
            tc.tile_pool(name="pt", bufs=3, space="PSUM") as ptpool,
            tc.tile_pool(name="acc", bufs=2, space="PSUM") as accpool,
            tc.tile_pool(name="warmps", bufs=1, space="PSUM") as wpool,
            tc.tile_pool(name="small", bufs=1) as spool,
        ):
            # ACT warm op: hoists the Square function-table load.
            warm = spool.tile([_P, 1], f32, tag="warm")
            nc.gpsimd.memset(warm[:], 0.0)
            nc.scalar.activation(warm[:], warm[:], SQ)

            # Constants built on-chip. iota with channel_multiplier=-1
            # gives v[p,j] = j - p; the stacked one-hot mask [128,64] has
            # ones where j - p is 0 or -64. Masks are pre-scaled so the
            # final combine needs no extra scaling: s-chain mask
            # sqrt(0.5)-one-hot -> s'^2 = 0.5 s^2; q-chain 0.5-one-hot.
            iot_i = cpool.tile([_P, _P], i32, tag="iot_i")
            iot_m = cpool.tile([_P, _E], i32, tag="iot_m")
            ident = cpool.tile([_P, _P], bf16, tag="ident")
            mask = cpool.tile([_P, _E], bf16, tag="mask")
            mask_b = cpool.tile([_P, _E], bf16, tag="mask_b")
            nc.gpsimd.iota(iot_i[:], pattern=[[1, _P]], base=0, channel_multiplier=-1)
            nc.gpsimd.iota(iot_m[:], pattern=[[1, _E]], base=0, channel_multiplier=-1)
            nc.vector.tensor_scalar(ident[:], iot_i[:], 0, None, op0=ALU.is_equal)
            nc.vector.tensor_scalar(mask[:], iot_m[:], 0, None, op0=ALU.is_equal)
            nc.vector.tensor_scalar(mask_b[:], iot_m[:], -_E, None, op0=ALU.is_equal)
            nc.vector.tensor_add(mask[:], mask[:], mask_b[:])
            maskh = cpool.tile([_P, _E], bf16, tag="maskh")
            maskq = cpool.tile([_P, _E], bf16, tag="maskq")
            nc.vector.tensor_scalar_mul(maskh[:], mask[:], HALF_SQRT)
            nc.vector.tensor_scalar_mul(maskq[:], mask[:], 0.5)

            # PE HAM warm-up: dummy matmuls on a zeroed tile, result never
            # read. Runs under the first input DMA's shadow and releases
            # the PE clock gate before the real transposes arrive.
            wmov = cpool.tile([_P, _P], bf16, tag="wmov")
            nc.gpsimd.memset(wmov[:], 0.0)
            # full-bank PSUM tile: start=True clears the whole bank, so no
            # other tile may share it
            wps = wpool.tile([_P, 512], f32, tag="wps")
            for _ in range(_WARM_MMS):
                nc.tensor.matmul(wps[:, :_P], wmov[:], ident[:], start=True, stop=True)

            for c in range(_NCHUNK):
                rows = slice(c * _P, (c + 1) * _P)
                slices = _SLICES[c]
                nblk_total = sum(n // 2 for n in slices)
                # full-bank PSUM accumulators (see wps comment)
                s_t = accpool.tile([_P, 512], f32, tag="s")
                q_t = accpool.tile([_P, 512], f32, tag="q")
                s_ps = s_t[:, :_E]
                q_ps = q_t[:, :_E]
                blk0 = 0
                col0 = 0
                for si, n in enumerate(slices):
                    w = n * _E
                    nblk = n // 2
                    xt = xpool.tile([_P, w], f32, tag=f"x{n}_{si}")
                    # alternate HWDGE rings (SP / ACT) so the two queues
                    # interleave at packet granularity and hide per-transfer
                    # handoff gaps on the SDMA rings
                    dma_eng = nc.sync if (c * 4 + si) % 2 == 0 else nc.scalar
                    dma_eng.dma_start(out=xt[:], in_=x[rows, col0 : col0 + w])
                    xb = xbpool.tile([_P, w], bf16, tag=f"xb{n}")
                    nc.vector.tensor_copy(xb[:], xt[:])
                    pt = ptpool.tile([_P, 8 * _P], bf16, tag="pt")
                    for j in range(nblk):
                        nc.tensor.transpose(
                            pt[:, j * _P : (j + 1) * _P],
                            xb[:, j * _P : (j + 1) * _P],
                            ident[:],
                        )
                    # copyback split between DVE and ACT for balance
                    xT = xtpool.tile([_P, w], bf16, tag=f"xT{n}")
                    h = (nblk // 2) * _P
                    if h > 0:
                        nc.vector.tensor_copy(xT[:, :h], pt[:, :h])
                    if h < w:
                        nc.scalar.activation(xT[:, h:w], pt[:, h:w], mybir.ActivationFunctionType.Copy)
                    sq = sqpool.tile([_P, w], bf16, tag=f"sq{n}")
                    nc.scalar.activation(sq[:], pt[:, :w], SQ)
                    # grouped matmuls: one stationary load per chain per slice
                    for j in range(nblk):
                        kk = blk0 + j
                        bcols = slice(j * _P, (j + 1) * _P)
                        nc.tensor.matmul(
                            s_ps,
                            xT[:, bcols],
                            maskh[:],
                            start=(kk == 0),
                            stop=(kk == nblk_total - 1),
                        )
                    for j in range(nblk):
                        kk = blk0 + j
                        bcols = slice(j * _P, (j + 1) * _P)
                        nc.tensor.matmul(
                            q_ps,
                            sq[:, bcols],
                            maskq[:],
                            start=(kk == 0),
                            stop=(kk == nblk_total - 1),
                        )
                    blk0 += nblk
                    col0 += w

                # res = 0.5*s^2 - 0.5*q
                m2 = spool.tile([_P, _E], f32, tag=f"m2_{c}")
                res = spool.tile([_P, _E], f32, tag=f"res_{c}")
                nc.scalar.activation(m2[:], s_ps, SQ)
                nc.vector.tensor_sub(res[:], m2[:], q_ps)
                nc.sync.dma_start(out=out[c], in_=res[:])
    nc.compile()
    return nc


_WALRUS_EXTRA = []


def _patch_walrus():
    """Hook to append extra walrus_driver args (e.g. --max-sem-num to cap
    the one-event-sem-op-per-semaphore zeroing postamble)."""
    from concourse import bass_utils

    if getattr(bass_utils, "_walrus_patched", False):
        return
    real_run = bass_utils.run_command

    def run2(cmd, **kw):
        if cmd and "walrus_driver" in str(cmd[0]):
            cmd = list(cmd) + _WALRUS_EXTRA
        return real_run(cmd, **kw)

    bass_utils.run_command = run2
    bass_utils._walrus_patched = True


def _run(in_maps, **kwargs):
    from concourse.bass_utils import run_bass_kernel_spmd

    _patch_walrus()
    nc = _build()
    return run_bass_kernel_spmd(nc, in_maps, core_ids=list(range(_NCORES)), **kwargs)


def _shard(inputs: np.ndarray):
    x = np.ascontiguousarray(
        np.asarray(inputs, dtype=np.float32).reshape(_B, _ROW)
    )
    return [
        {"inputs": np.ascontiguousarray(x[i * _BS : (i + 1) * _BS])}
        for i in range(_NCORES)
    ]


def kernel(
    inputs: np.ndarray,
    weight_attention: np.ndarray = None,
    weight_projection: np.ndarray = None,
    weight_bias: np.ndarray = None,
) -> np.ndarray:
    # weights are dead code (softmax over a size-1 axis == 1.0)
    res = _run(_shard(inputs))
    return np.concatenate(
        [r["out"].reshape(_BS, _E) for r in res.results], axis=0
    )


# revision 17
# speedup vs baseline: 1.1255x; 1.0153x over previous
"""Trainium2 Bass kernel for nn_AttentionLayer_77558519431766.

Math: the reference computes softmax over a size-1 axis, which is
identically 1.0, so the attention MLP is dead code and

    out[b, e] = sum_{i<j} x[b,i,e] * x[b,j,e]
              = 0.5 * ((sum_f x[b,f,e])^2 - sum_f x[b,f,e]^2)

Implementation v4 (PE reduction pipeline, column-sliced DMA):
  Per 128-row chunk the 50 f-planes stream in as column slices so compute
  starts ~4us in instead of waiting for the whole chunk. Per slice:
    - DVE casts f32 -> bf16 (2x_2p)
    - PE transposes each [128b, 128(f2,e)] block into PSUM
    - DVE + ACT split the PSUM->SBUF copyback (xT for the s-chain);
      ACT squares the transposed blocks into SBUF (sq, scaled by 0.5)
    - PE accumulates s = sum_f x and q = 0.5*sum_f x^2 via one-hot
      stacked masks [128,64], grouped per slice to limit LDWEIGHTS churn
  Chunk combine: res = Square(s*sqrt(.5)) - q, one small DMA out.
  ~30 dummy matmuls at t=0 warm the PE HAM clock gate (1.2 -> 2.4 GHz)
  under the first DMA's shadow.

Sharding: pure data parallelism, batch 2048 -> 8 shards of 256.
"""

import numpy as np

try:
    import concourse.bass as bass  # noqa: F401
except ImportError:  # pragma: no cover
    import sys

    sys.path.insert(0, "/opt/trn_rl_repo")

_B, _F, _E = 2048, 50, 64
_NCORES = 8
_BS = _B // _NCORES  # 256 rows per core
_ROW = _F * _E  # 3200 floats per row
_P = 128  # SBUF partitions
_NCHUNK = _BS // _P  # 2
# f-planes per DMA slice, per chunk (must be even: 1 block = 2 planes).
# Last slice of the last chunk is small to shorten the critical tail.
_SLICES = [
    [26, 24],
    [26, 20, 4],
]
_WARM_MMS = 30  # dummy matmuls to release the PE HAM clock gate


def _make_tc_class():
    """TileContext with a slim kernel tail (drops the redundant tail
    sem-clear + second barrier; the Bass preamble re-clears at start)."""
    from concourse.tile import TileContext
    from concourse.vector_clock import ScopedClock

    class SlimTailTileContext(TileContext):
        def _drain_and_barrier(self, tick_clock, wait_clock):
            drain_inst = self.nc.sync.drain()
            wait_clock.add_sem_waits(
                drain_inst.ins, ScopedClock({None: tick_clock.global_clock})
            )
            self.nc.all_engine_barrier(sem_only=True)
            popped = self.nc._tile_sem_poison_stack.pop()
            assert popped is self._sem_poison

    return SlimTailTileContext


def _build():
    import concourse.bacc as bacc
    import concourse.mybir as mybir

    TileContext = _make_tc_class()

    f32 = mybir.dt.float32
    bf16 = mybir.dt.bfloat16
    i32 = mybir.dt.int32
    SQ = mybir.ActivationFunctionType.Square
    ALU = mybir.AluOpType
    HALF_SQRT = float(np.float32(np.sqrt(0.5)))

    nc = bacc.Bacc()
    x = nc.declare_dram_parameter("inputs", [_BS, _ROW], f32, isOutput=False)
    out = nc.declare_dram_parameter("out", [_NCHUNK, _P, _E], f32, isOutput=True)

    with TileContext(nc) as tc:
        with (
            tc.tile_pool(name="consts", bufs=1) as cpool,
            tc.tile_pool(name="x", bufs=6) as xpool,
            tc.tile_pool(name="xb", bufs=3) as xbpool,
            tc.tile_pool(name="xT", bufs=3) as xtpool,
            tc.tile_pool(name="xsq", bufs=3) as sqpool,
            tc.tile_pool(name="pt", bufs=3, space="PSUM") as ptpool,
            tc.tile_pool(name="acc", bufs=2, space="PSUM") as accpool,
            tc.tile_pool(name="warmps", bufs=1, space="PSUM") as wpool,
            tc.tile_pool(name="small", bufs=1) as spool,
        ):
            # ACT warm op: hoists the Square function-table load.
            warm = spool.tile([_P, 1], f32, tag="warm")
            nc.gpsimd.memset(warm[:], 0.0)
            nc.scalar.activation(warm[:], warm[:], SQ)

            # Constants built on-chip. iota with channel_multiplier=-1
            # gives v[p,j] = j - p; the stacked one-hot mask [128,64] has
            # ones where j - p is 0 or -64. Masks are pre-scaled so the
            # final combine needs no extra scaling: s-chain mask
            # sqrt(0.5)-one-hot -> s'^2 = 0.5 s^2; q-chain 0.5-one-hot.
            iot_i = cpool.tile([_P, _P], i32, tag="iot_i")
            iot_m = cpool.tile([_P, _E], i32, tag="iot_m")
            ident = cpool.tile([_P, _P], bf16, tag="ident")
            mask = cpool.tile([_P, _E], bf16, tag="mask")
            mask_b = cpool.tile([_P, _E], bf16, tag="mask_b")
            nc.gpsimd.iota(iot_i[:], pattern=[[1, _P]], base=0, channel_multiplier=-1)
            nc.gpsimd.iota(iot_m[:], pattern=[[1, _E]], base=0, channel_multiplier=-1)
            nc.vector.tensor_scalar(ident[:], iot_i[:], 0, None, op0=ALU.is_equal)
            nc.vector.tensor_scalar(mask[:], iot_m[:], 0, None, op0=ALU.is_equal)
            nc.vector.tensor_scalar(mask_b[:], iot_m[:], -_E, None, op0=ALU.is_equal)
            nc.vector.tensor_add(mask[:], mask[:], mask_b[:])
            maskh = cpool.tile([_P, _E], bf16, tag="maskh")
            maskq = cpool.tile([_P, _E], bf16, tag="maskq")
            nc.vector.tensor_scalar_mul(maskh[:], mask[:], HALF_SQRT)
            nc.vector.tensor_scalar_mul(maskq[:], mask[:], 0.5)

            # PE HAM warm-up: dummy matmuls on a zeroed tile, result never
            # read. Runs under the first input DMA's shadow and releases
            # the PE clock gate before the real transposes arrive.
            wmov = cpool.tile([_P, _P], bf16, tag="wmov")
            nc.gpsimd.memset(wmov[:], 0.0)
            # full-bank PSUM tile: start=True clears the whole bank, so no
            # other tile may share it
            wps = wpool.tile([_P, 512], f32, tag="wps")
            for _ in range(_WARM_MMS):
                nc.tensor.matmul(wps[:, :_P], wmov[:], ident[:], start=True, stop=True)

            for c in range(_NCHUNK):
                rows = slice(c * _P, (c + 1) * _P)
                slices = _SLICES[c]
                nblk_total = sum(n // 2 for n in slices)
                # full-bank PSUM accumulators (start=True clears the bank)
                s_t = accpool.tile([_P, 512], f32, tag="s")
                q_t = accpool.tile([_P, 512], f32, tag="q")
                s_ps = s_t[:, :_E]
                q_ps = q_t[:, :_E]
                blk0 = 0
                col0 = 0
                for si, n in enumerate(slices):
                    w = n * _E
                    nblk = n // 2
                    xt = xpool.tile([_P, w], f32, tag=f"x{n}_{si}")
                    nc.sync.dma_start(out=xt[:], in_=x[rows, col0 : col0 + w])
                    # compute in groups of <=8 blocks (one PSUM bank each)
                    g0 = 0
                    while g0 < nblk:
                        gn = min(8, nblk - g0)
                        gw = gn * _P
                        gcols = slice(g0 * _P, (g0 + gn) * _P)
                        xb = xbpool.tile([_P, 8 * _P], bf16, tag="xb")
                        nc.vector.tensor_copy(xb[:, :gw], xt[:, gcols])
                        pt = ptpool.tile([_P, 8 * _P], bf16, tag="pt")
                        for j in range(gn):
                            nc.tensor.transpose(
                                pt[:, j * _P : (j + 1) * _P],
                                xb[:, j * _P : (j + 1) * _P],
                                ident[:],
                            )
                        # copyback on ACT, squares on DVE (2x tensor_tensor)
                        xT = xtpool.tile([_P, 8 * _P], bf16, tag="xT")
                        nc.scalar.activation(
                            xT[:, :gw], pt[:, :gw],
                            mybir.ActivationFunctionType.Copy,
                        )
                        sq = sqpool.tile([_P, 8 * _P], bf16, tag="sq")
                        nc.vector.tensor_tensor(
                            sq[:, :gw], xT[:, :gw], pt[:, :gw],
                            op=ALU.mult,
                        )
                        for j in range(gn):
                            kk = blk0 + g0 + j
                            bcols = slice(j * _P, (j + 1) * _P)
                            nc.tensor.matmul(
                                s_ps,
                                xT[:, bcols],
                                maskh[:],
                                start=(kk == 0),
                                stop=(kk == nblk_total - 1),
                            )
                        for j in range(gn):
                            kk = blk0 + g0 + j
                            bcols = slice(j * _P, (j + 1) * _P)
                            nc.tensor.matmul(
                                q_ps,
                                sq[:, bcols],
                                maskq[:],
                                start=(kk == 0),
                                stop=(kk == nblk_total - 1),
                            )
                        g0 += gn
                    blk0 += nblk
                    col0 += w

                # res = 0.5*s^2 - 0.5*q
                m2 = spool.tile([_P, _E], f32, tag=f"m2_{c}")
                res = spool.tile([_P, _E], f32, tag=f"res_{c}")
                nc.scalar.activation(m2[:], s_ps, SQ)
                nc.vector.tensor_sub(res[:], m2[:], q_ps)
                nc.sync.dma_start(out=out[c], in_=res[:])
    nc.compile()
    return nc


_WALRUS_EXTRA = []


def _patch_walrus():
    """Hook to append extra walrus_driver args (e.g. --max-sem-num to cap
    the one-event-sem-op-per-semaphore zeroing postamble)."""
    from concourse import bass_utils

    if getattr(bass_utils, "_walrus_patched", False):
        return
    real_run = bass_utils.run_command

    def run2(cmd, **kw):
        if cmd and "walrus_driver" in str(cmd[0]):
            cmd = list(cmd) + _WALRUS_EXTRA
        return real_run(cmd, **kw)

    bass_utils.run_command = run2
    bass_utils._walrus_patched = True


def _run(in_maps, **kwargs):
    from concourse.bass_utils import run_bass_kernel_spmd

    _patch_walrus()
    nc = _build()
    return run_bass_kernel_spmd(nc, in_maps, core_ids=list(range(_NCORES)), **kwargs)


def _shard(inputs: np.ndarray):
    x = np.ascontiguousarray(
        np.asarray(inputs, dtype=np.float32).reshape(_B, _ROW)
    )
    return [
        {"inputs": np.ascontiguousarray(x[i * _BS : (i + 1) * _BS])}
        for i in range(_NCORES)
    ]


def kernel(
    inputs: np.ndarray,
    weight_attention: np.ndarray = None,
    weight_projection: np.ndarray = None,
    weight_bias: np.ndarray = None,
) -> np.ndarray:
    # weights are dead code (softmax over a size-1 axis == 1.0)
    res = _run(_shard(inputs))
    return np.concatenate(
        [r["out"].reshape(_BS, _E) for r in res.results], axis=0
    )


# revision 19
# speedup vs baseline: 1.2634x; 1.1226x over previous
"""Trainium2 Bass kernel for nn_AttentionLayer_77558519431766.

Math: the reference computes softmax over a size-1 axis, which is
identically 1.0, so the attention MLP is dead code and

    out[b, e] = sum_{i<j} x[b,i,e] * x[b,j,e]
              = 0.5 * ((sum_f x[b,f,e])^2 - sum_f x[b,f,e]^2)

Implementation (per 128-sample chunk, layout [128b, f*64+e]):
  1. ACT casts x to bf16.
  2. PE transposes each [128b, 128(f2,e)] block into PSUM, so pairs of
     f-rows land on partitions.
  3. DVE copies the transposed blocks back to SBUF (for s), ACT squares
     them into SBUF (for q).
  4. PE runs two matmul accumulation chains against a stacked-identity
     mask [128,64] (row (f2,e) is one-hot at e), yielding
     s = sum_f x and q = sum_f x^2 as [128b, 64e] in PSUM.
  5. res = 0.5*s^2 - 0.5*q, DMA out.

Sharding: pure data parallelism, batch 2048 -> 8 shards of 256.
"""

import numpy as np

try:
    import concourse.bass as bass  # noqa: F401
except ImportError:  # pragma: no cover
    import sys

    sys.path.insert(0, "/opt/trn_rl_repo")

_B, _F, _E = 2048, 50, 64
_NCORES = 8
_BS = _B // _NCORES  # 256 rows per core
_ROW = _F * _E  # 3200 floats per row
_P = 128  # SBUF partitions
_NBLK = _ROW // _P  # 25 transpose blocks per chunk


def _make_tc_class():
    """TileContext with a slim kernel tail.

    Stock TileContext ends with drain -> full all-engine barrier ->
    semaphore clear -> second full barrier (~6-8us of EVSEM butterfly).
    The Bass preamble already dma_reset+sem_clears the entire kernel
    semaphore range at the start of every execution, so the tail clear
    and second barrier are redundant for a single-TileContext kernel.
    Keep the global-clock drain (output DMA completion) plus one cheap
    sequencer-level barrier.
    """
    from concourse.tile import TileContext
    from concourse.vector_clock import ScopedClock

    class SlimTailTileContext(TileContext):
        def _drain_and_barrier(self, tick_clock, wait_clock):
            drain_inst = self.nc.sync.drain()
            wait_clock.add_sem_waits(
                drain_inst.ins, ScopedClock({None: tick_clock.global_clock})
            )
            self.nc.all_engine_barrier(sem_only=True)
            popped = self.nc._tile_sem_poison_stack.pop()
            assert popped is self._sem_poison

    return SlimTailTileContext


def _build():
    import concourse.bacc as bacc
    import concourse.mybir as mybir

    TileContext = _make_tc_class()

    f32 = mybir.dt.float32
    bf16 = mybir.dt.bfloat16
    SQ = mybir.ActivationFunctionType.Square
    CP = mybir.ActivationFunctionType.Copy
    ALU = mybir.AluOpType
    HALF_SQRT = float(np.float32(np.sqrt(0.5)))

    i32 = mybir.dt.int32

    nc = bacc.Bacc()
    x = nc.declare_dram_parameter("inputs", [_BS, _ROW], f32, isOutput=False)
    out = nc.declare_dram_parameter("out", [_BS, _E], f32, isOutput=True)

    n_chunks = _BS // _P  # 2
    # two DMAs per chunk (13.3KB/11.3KB row packets) so compute starts on
    # the first half ~3us before the full chunk has landed
    halves = [(0, 13), (13, 12)]
    groups_by_half = {13: [7, 6], 12: [7, 5]}

    with TileContext(nc) as tc:
        with (
            tc.tile_pool(name="consts", bufs=1) as cpool,
            tc.tile_pool(name="x", bufs=4) as xpool,
            tc.tile_pool(name="xb", bufs=4) as xbpool,
            tc.tile_pool(name="xT", bufs=4) as xtpool,
            tc.tile_pool(name="xsq", bufs=4) as sqpool,
            tc.tile_pool(name="pt", bufs=3, space="PSUM") as ptpool,
            tc.tile_pool(name="acc", bufs=2, space="PSUM") as accpool,
            tc.tile_pool(name="warmps", bufs=1, space="PSUM") as wpool,
            tc.tile_pool(name="small", bufs=2) as spool,
        ):
            # Warm op: forces the ACT function-table load off the critical
            # path (it otherwise lands right before the first Square, after
            # a cross-engine wait).
            warm = spool.tile([_P, 1], f32, tag="warm")
            nc.gpsimd.memset(warm[:], 0.0)
            nc.scalar.activation(warm[:], warm[:], SQ)

            # Constants built on-chip (a DMA for these queues behind the
            # input packets and stalls the first transposes by multiple us).
            # iota with channel_multiplier=-1 gives v[p,j] = j - p, so
            # identity = (v == 0); the stacked mask [128,64] has ones where
            # j - p is 0 or -64.
            iot_i = cpool.tile([_P, _P], i32, tag="iot_i")
            iot_m = cpool.tile([_P, _E], i32, tag="iot_m")
            ident = cpool.tile([_P, _P], bf16, tag="ident")
            mask = cpool.tile([_P, _E], bf16, tag="mask")
            mask_b = cpool.tile([_P, _E], bf16, tag="mask_b")
            nc.gpsimd.iota(iot_i[:], pattern=[[1, _P]], base=0, channel_multiplier=-1)
            nc.gpsimd.iota(iot_m[:], pattern=[[1, _E]], base=0, channel_multiplier=-1)
            nc.vector.tensor_scalar(
                ident[:], iot_i[:], 0, None, op0=ALU.is_equal
            )
            nc.vector.tensor_scalar(
                mask[:], iot_m[:], 0, None, op0=ALU.is_equal
            )
            nc.vector.tensor_scalar(
                mask_b[:], iot_m[:], -_E, None, op0=ALU.is_equal
            )
            nc.vector.tensor_add(mask[:], mask[:], mask_b[:])
            # Pre-scale the chain masks so the combine needs no scaling:
            # s-chain mask = sqrt(0.5)*one-hot -> s'^2 = 0.5*s^2 (up to the
            # bf16 rounding of sqrt(0.5): (c^2-0.5)*s^2 ~ 1e-4 rel, noise
            # next to the 2.4e-3 bf16-squares error); q-chain mask =
            # 0.5*one-hot (exact in bf16) -> q' = 0.5*q.
            maskh = cpool.tile([_P, _E], bf16, tag="maskh")
            maskq = cpool.tile([_P, _E], bf16, tag="maskq")
            nc.vector.tensor_scalar_mul(maskh[:], mask[:], HALF_SQRT)
            nc.vector.tensor_scalar_mul(maskq[:], mask[:], 0.5)

            # PE HAM warm-up: dummy matmuls under the first DMA's shadow
            # release the PE clock gate (1.2 -> 2.4 GHz) before real work.
            wmov = cpool.tile([_P, _P], bf16, tag="wmov")
            nc.gpsimd.memset(wmov[:], 0.0)
            wps = wpool.tile([_P, 512], f32, tag="wps")
            for _ in range(30):
                nc.tensor.matmul(wps[:, :_P], wmov[:], ident[:], start=True, stop=True)

            for c in range(n_chunks):
                rows = slice(c * _P, (c + 1) * _P)
                # separate banks: a start=True matmul clears its whole bank,
                # so the two accumulation chains must not share one
                s_t = accpool.tile([_P, 512], f32, tag="s")
                q_t = accpool.tile([_P, 512], f32, tag="q")
                s_ps = s_t[:, :_E]
                q_ps = q_t[:, :_E]
                for blk0, nblk in halves:
                    cols = slice(blk0 * _P, (blk0 + nblk) * _P)
                    n = nblk * _P
                    xt = xpool.tile([_P, n], f32, tag="x")
                    nc.sync.dma_start(out=xt[:], in_=x[rows, cols])
                    xbt = xbpool.tile([_P, n], bf16, tag="xb")

                    xT = xtpool.tile([_P, n], bf16, tag="xT")
                    xsq = sqpool.tile([_P, n], bf16, tag="xsq")
                    g0 = 0
                    for gn in groups_by_half[nblk]:
                        gcols = slice(g0 * _P, (g0 + gn) * _P)
                        # per-group cast: lets the tail chunk pipeline at
                        # group granularity (gpsimd CAST measured 4x slower
                        # than DVE's 2x mode, so this stays on DVE)
                        nc.vector.tensor_copy(xbt[:, gcols], xt[:, gcols])
                        pt = ptpool.tile([_P, 7 * _P], bf16, tag="pt")
                        for j in range(gn):
                            k = g0 + j
                            nc.tensor.transpose(
                                pt[:, j * _P : (j + 1) * _P],
                                xbt[:, k * _P : (k + 1) * _P],
                                ident[:],
                            )
                        nc.vector.tensor_copy(xT[:, gcols], pt[:, : gn * _P])
                        nc.scalar.activation(xsq[:, gcols], pt[:, : gn * _P], SQ)
                        g0 += gn
                    for k in range(nblk):
                        kk = blk0 + k
                        bcols = slice(k * _P, (k + 1) * _P)
                        nc.tensor.matmul(
                            s_ps,
                            xT[:, bcols],
                            maskh[:],
                            start=(kk == 0),
                            stop=(kk == _NBLK - 1),
                        )
                        nc.tensor.matmul(
                            q_ps,
                            xsq[:, bcols],
                            maskq[:],
                            start=(kk == 0),
                            stop=(kk == _NBLK - 1),
                        )

                # res = s'^2 - q' = 0.5*s^2 - 0.5*q
                m2 = spool.tile([_P, _E], f32, tag="m2")
                res = spool.tile([_P, _E], f32, tag="res")
                nc.scalar.activation(m2[:], s_ps, SQ)
                nc.vector.tensor_sub(res[:], m2[:], q_ps)
                nc.sync.dma_start(out=out[rows, :], in_=res[:])
    nc.compile()
    return nc


_WALRUS_EXTRA = []


def _patch_walrus():
    """Cap walrus's semaphore allocation: the NEFF postamble zeroes every
    allocated semaphore one event-sem op at a time (spaced to dodge the
    event-accel erratum), so unused semaphores cost ~150ns each at the
    kernel tail."""
    from concourse import bass_utils

    if getattr(bass_utils, "_walrus_patched", False):
        return
    real_run = bass_utils.run_command

    def run2(cmd, **kw):
        if cmd and "walrus_driver" in str(cmd[0]):
            cmd = list(cmd) + _WALRUS_EXTRA
        return real_run(cmd, **kw)

    bass_utils.run_command = run2
    bass_utils._walrus_patched = True


def _run(in_maps, **kwargs):
    from concourse.bass_utils import run_bass_kernel_spmd

    _patch_walrus()
    nc = _build()
    return run_bass_kernel_spmd(nc, in_maps, core_ids=list(range(_NCORES)), **kwargs)


def _shard(inputs: np.ndarray):
    x = np.ascontiguousarray(
        np.asarray(inputs, dtype=np.float32).reshape(_B, _ROW)
    )
    return [
        {"inputs": np.ascontiguousarray(x[i * _BS : (i + 1) * _BS])}
        for i in range(_NCORES)
    ]


def kernel(
    inputs: np.ndarray,
    weight_attention: np.ndarray = None,
    weight_projection: np.ndarray = None,
    weight_bias: np.ndarray = None,
) -> np.ndarray:
    # weights are dead code (softmax over a size-1 axis == 1.0)
    res = _run(_shard(inputs))
    return np.concatenate([r["out"] for r in res.results], axis=0)



# revision 22
# speedup vs baseline: 1.2848x; 1.0169x over previous
"""Trainium2 Bass kernel for nn_AttentionLayer_77558519431766.

Math: the reference computes softmax over a size-1 axis, which is
identically 1.0, so the attention MLP is dead code and

    out[b, e] = sum_{i<j} x[b,i,e] * x[b,j,e]
              = 0.5 * ((sum_f x[b,f,e])^2 - sum_f x[b,f,e]^2)

Implementation (per 128-sample chunk, layout [128b, f*64+e]):
  1. ACT casts x to bf16.
  2. PE transposes each [128b, 128(f2,e)] block into PSUM, so pairs of
     f-rows land on partitions.
  3. DVE copies the transposed blocks back to SBUF (for s), ACT squares
     them into SBUF (for q).
  4. PE runs two matmul accumulation chains against a stacked-identity
     mask [128,64] (row (f2,e) is one-hot at e), yielding
     s = sum_f x and q = sum_f x^2 as [128b, 64e] in PSUM.
  5. res = 0.5*s^2 - 0.5*q, DMA out.

Sharding: pure data parallelism, batch 2048 -> 8 shards of 256.
"""

import numpy as np

try:
    import concourse.bass as bass  # noqa: F401
except ImportError:  # pragma: no cover
    import sys

    sys.path.insert(0, "/opt/trn_rl_repo")

_B, _F, _E = 2048, 50, 64
_NCORES = 8
_BS = _B // _NCORES  # 256 rows per core
_ROW = _F * _E  # 3200 floats per row
_P = 128  # SBUF partitions
_NBLK = _ROW // _P  # 25 transpose blocks per chunk


def _make_tc_class():
    """TileContext with a slim kernel tail.

    Stock TileContext ends with drain -> full all-engine barrier ->
    semaphore clear -> second full barrier (~6-8us of EVSEM butterfly).
    The Bass preamble already dma_reset+sem_clears the entire kernel
    semaphore range at the start of every execution, so the tail clear
    and second barrier are redundant for a single-TileContext kernel.
    Keep the global-clock drain (output DMA completion) plus one cheap
    sequencer-level barrier.
    """
    from concourse.tile import TileContext
    from concourse.vector_clock import ScopedClock

    class SlimTailTileContext(TileContext):
        def _drain_and_barrier(self, tick_clock, wait_clock):
            drain_inst = self.nc.sync.drain()
            wait_clock.add_sem_waits(
                drain_inst.ins, ScopedClock({None: tick_clock.global_clock})
            )
            self.nc.all_engine_barrier(sem_only=True)
            popped = self.nc._tile_sem_poison_stack.pop()
            assert popped is self._sem_poison

    return SlimTailTileContext


def _build():
    import concourse.bacc as bacc
    import concourse.mybir as mybir

    TileContext = _make_tc_class()

    f32 = mybir.dt.float32
    bf16 = mybir.dt.bfloat16
    SQ = mybir.ActivationFunctionType.Square
    CP = mybir.ActivationFunctionType.Copy
    ALU = mybir.AluOpType
    HALF_SQRT = float(np.float32(np.sqrt(0.5)))

    i32 = mybir.dt.int32

    nc = bacc.Bacc()
    x = nc.declare_dram_parameter("inputs", [_BS, _ROW], f32, isOutput=False)
    out = nc.declare_dram_parameter("out", [_BS, _E], f32, isOutput=True)

    n_chunks = _BS // _P  # 2
    # DMA split per chunk: small first slice for an early compute start,
    # tiny last slice on the final chunk to shorten the critical tail
    halves_by_chunk = [
        [(0, 13), (13, 12)],
        [(0, 13), (13, 12)],
    ]
    groups_by_half = {13: [7, 6], 12: [7, 5]}

    with TileContext(nc) as tc:
        with (
            tc.tile_pool(name="consts", bufs=1) as cpool,
            tc.tile_pool(name="x", bufs=4) as xpool,
            tc.tile_pool(name="xb", bufs=4) as xbpool,
            tc.tile_pool(name="xT", bufs=4) as xtpool,
            tc.tile_pool(name="xsq", bufs=4) as sqpool,
            tc.tile_pool(name="pt", bufs=3, space="PSUM") as ptpool,
            tc.tile_pool(name="acc", bufs=2, space="PSUM") as accpool,
            tc.tile_pool(name="warmps", bufs=1, space="PSUM") as wpool,
            tc.tile_pool(name="small", bufs=2) as spool,
        ):
            # Warm op: forces the ACT function-table load off the critical
            # path (it otherwise lands right before the first Square, after
            # a cross-engine wait).
            warm = spool.tile([_P, 1], f32, tag="warm")
            nc.gpsimd.memset(warm[:], 0.0)
            nc.scalar.activation(warm[:], warm[:], SQ)

            # Constants built on-chip (a DMA for these queues behind the
            # input packets and stalls the first transposes by multiple us).
            # iota with channel_multiplier=-1 gives v[p,j] = j - p, so
            # identity = (v == 0); the stacked mask [128,64] has ones where
            # j - p is 0 or -64.
            iot_i = cpool.tile([_P, _P], i32, tag="iot_i")
            iot_m = cpool.tile([_P, _E], i32, tag="iot_m")
            ident = cpool.tile([_P, _P], bf16, tag="ident")
            mask = cpool.tile([_P, _E], bf16, tag="mask")
            mask_b = cpool.tile([_P, _E], bf16, tag="mask_b")
            nc.gpsimd.iota(iot_i[:], pattern=[[1, _P]], base=0, channel_multiplier=-1)
            nc.gpsimd.iota(iot_m[:], pattern=[[1, _E]], base=0, channel_multiplier=-1)
            nc.vector.tensor_scalar(
                ident[:], iot_i[:], 0, None, op0=ALU.is_equal
            )
            nc.vector.tensor_scalar(
                mask[:], iot_m[:], 0, None, op0=ALU.is_equal
            )
            nc.vector.tensor_scalar(
                mask_b[:], iot_m[:], -_E, None, op0=ALU.is_equal
            )
            nc.vector.tensor_add(mask[:], mask[:], mask_b[:])
            # Pre-scale the chain masks so the combine needs no scaling:
            # s-chain mask = sqrt(0.5)*one-hot -> s'^2 = 0.5*s^2 (up to the
            # bf16 rounding of sqrt(0.5): (c^2-0.5)*s^2 ~ 1e-4 rel, noise
            # next to the 2.4e-3 bf16-squares error); q-chain mask =
            # 0.5*one-hot (exact in bf16) -> q' = 0.5*q.
            maskh = cpool.tile([_P, _E], bf16, tag="maskh")
            maskq = cpool.tile([_P, _E], bf16, tag="maskq")
            nc.vector.tensor_scalar_mul(maskh[:], mask[:], HALF_SQRT)
            nc.vector.tensor_scalar_mul(maskq[:], mask[:], 0.5)

            # PE HAM warm-up: dummy matmuls under the first DMA's shadow
            # release the PE clock gate (1.2 -> 2.4 GHz) before real work.
            wmov = cpool.tile([_P, _P], bf16, tag="wmov")
            nc.gpsimd.memset(wmov[:], 0.0)
            wps = wpool.tile([_P, 512], f32, tag="wps")
            for _ in range(30):
                nc.tensor.matmul(wps[:, :_P], wmov[:], ident[:], start=True, stop=True)

            for c in range(n_chunks):
                rows = slice(c * _P, (c + 1) * _P)
                # separate banks: a start=True matmul clears its whole bank,
                # so the two accumulation chains must not share one
                s_t = accpool.tile([_P, 512], f32, tag="s")
                q_t = accpool.tile([_P, 512], f32, tag="q")
                s_ps = s_t[:, :_E]
                q_ps = q_t[:, :_E]
                for blk0, nblk in halves_by_chunk[c]:
                    cols = slice(blk0 * _P, (blk0 + nblk) * _P)
                    n = nblk * _P
                    xt = xpool.tile([_P, n], f32, tag="x")
                    nc.sync.dma_start(out=xt[:], in_=x[rows, cols])
                    xbt = xbpool.tile([_P, n], bf16, tag="xb")

                    xT = xtpool.tile([_P, n], bf16, tag="xT")
                    xsq = sqpool.tile([_P, n], bf16, tag="xsq")
                    g0 = 0
                    for gn in groups_by_half[nblk]:
                        gcols = slice(g0 * _P, (g0 + gn) * _P)
                        # per-group cast: lets the tail chunk pipeline at
                        # group granularity (gpsimd CAST measured 4x slower
                        # than DVE's 2x mode, so this stays on DVE)
                        nc.vector.tensor_copy(xbt[:, gcols], xt[:, gcols])
                        pt = ptpool.tile([_P, 7 * _P], bf16, tag="pt")
                        for j in range(gn):
                            k = g0 + j
                            nc.tensor.transpose(
                                pt[:, j * _P : (j + 1) * _P],
                                xbt[:, k * _P : (k + 1) * _P],
                                ident[:],
                            )
                        nc.vector.tensor_copy(xT[:, gcols], pt[:, : gn * _P])
                        nc.scalar.activation(xsq[:, gcols], pt[:, : gn * _P], SQ)
                        g0 += gn
                    for k in range(nblk):
                        kk = blk0 + k
                        bcols = slice(k * _P, (k + 1) * _P)
                        nc.tensor.matmul(
                            s_ps,
                            xT[:, bcols],
                            maskh[:],
                            start=(kk == 0),
                            stop=(kk == _NBLK - 1),
                        )
                        nc.tensor.matmul(
                            q_ps,
                            xsq[:, bcols],
                            maskq[:],
                            start=(kk == 0),
                            stop=(kk == _NBLK - 1),
                        )

                # res = s'^2 - q' = 0.5*s^2 - 0.5*q
                m2 = spool.tile([_P, _E], f32, tag="m2")
                res = spool.tile([_P, _E], f32, tag="res")
                nc.scalar.activation(m2[:], s_ps, SQ)
                nc.vector.tensor_sub(res[:], m2[:], q_ps)
                nc.sync.dma_start(out=out[rows, :], in_=res[:])
    nc.compile()
    return nc


_WALRUS_EXTRA = []


def _patch_walrus():
    """Cap walrus's semaphore allocation: the NEFF postamble zeroes every
    allocated semaphore one event-sem op at a time (spaced to dodge the
    event-accel erratum), so unused semaphores cost ~150ns each at the
    kernel tail."""
    from concourse import bass_utils

    if getattr(bass_utils, "_walrus_patched", False):
        return
    real_run = bass_utils.run_command

    def run2(cmd, **kw):
        if cmd and "walrus_driver" in str(cmd[0]):
            cmd = list(cmd) + _WALRUS_EXTRA
        return real_run(cmd, **kw)

    bass_utils.run_command = run2
    bass_utils._walrus_patched = True


def _run(in_maps, **kwargs):
    from concourse.bass_utils import run_bass_kernel_spmd

    _patch_walrus()
    nc = _build()
    return run_bass_kernel_spmd(nc, in_maps, core_ids=list(range(_NCORES)), **kwargs)


def _shard(inputs: np.ndarray):
    x = np.ascontiguousarray(
        np.asarray(inputs, dtype=np.float32).reshape(_B, _ROW)
    )
    return [
        {"inputs": np.ascontiguousarray(x[i * _BS : (i + 1) * _BS])}
        for i in range(_NCORES)
    ]


def kernel(
    inputs: np.ndarray,
    weight_attention: np.ndarray = None,
    weight_projection: np.ndarray = None,
    weight_bias: np.ndarray = None,
) -> np.ndarray:
    # weights are dead code (softmax over a size-1 axis == 1.0)
    res = _run(_shard(inputs))
    return np.concatenate([r["out"] for r in res.results], axis=0)



# revision 30
# speedup vs baseline: 1.3004x; 1.0122x over previous
"""Trainium2 Bass kernel for nn_AttentionLayer_77558519431766.

Math: the reference computes softmax over a size-1 axis, which is
identically 1.0, so the attention MLP is dead code and

    out[b, e] = sum_{i<j} x[b,i,e] * x[b,j,e]
              = 0.5 * ((sum_f x[b,f,e])^2 - sum_f x[b,f,e]^2)

Implementation (per 128-sample chunk, layout [128b, f*64+e]):
  1. Each chunk's input streams in as TWO column-sliced DMAs (11/14 and
     13/12 transpose-blocks; 11-13KB row packets keep ~293 GB/s) so
     compute starts ~3us before the full chunk has landed.
  2. ~45 dummy matmuls at t=0 (under the first DMA's shadow) release the
     PE HAM clock gate (1.2 -> 2.4 GHz) before the real work arrives.
  3. Per block-group (<=7 blocks = one PSUM bank): DVE casts to bf16
     (2x_2p), PE transposes each [128b, 128(f2,e)] block into PSUM, DVE
     copies the transposed blocks back to SBUF (s path), ACT squares them
     into SBUF (q path).
  4. PE runs two matmul accumulation chains against a stacked-identity
     mask [128,64] (row (f2,e) is one-hot at e), yielding s = sum_f x and
     q = sum_f x^2 as [128b, 64e] in PSUM. Full-bank accumulators: a
     start=True matmul clears its whole bank, so s/q/warm never share.
  5. res = 0.5*s^2 - 0.5*q (mask pre-scaling), DMA out.

Sharding: pure data parallelism, batch 2048 -> 8 shards of 256.
Measured: ~28.0-28.6us on 8 cores (baseline 29.8-30.6); rel err 2.4e-3
(bf16 squares; threshold 2e-2).
"""

import numpy as np

try:
    import concourse.bass as bass  # noqa: F401
except ImportError:  # pragma: no cover
    import sys

    sys.path.insert(0, "/opt/trn_rl_repo")

_B, _F, _E = 2048, 50, 64
_NCORES = 8
_BS = _B // _NCORES  # 256 rows per core
_ROW = _F * _E  # 3200 floats per row
_P = 128  # SBUF partitions
_NBLK = _ROW // _P  # 25 transpose blocks per chunk


def _make_tc_class():
    """TileContext with a slim kernel tail.

    Stock TileContext ends with drain -> full all-engine barrier ->
    semaphore clear -> second full barrier (~6-8us of EVSEM butterfly).
    The Bass preamble already dma_reset+sem_clears the entire kernel
    semaphore range at the start of every execution, so the tail clear
    and second barrier are redundant for a single-TileContext kernel.
    Keep the global-clock drain (output DMA completion) plus one cheap
    sequencer-level barrier.
    """
    from concourse.tile import TileContext
    from concourse.vector_clock import ScopedClock

    class SlimTailTileContext(TileContext):
        def _drain_and_barrier(self, tick_clock, wait_clock):
            drain_inst = self.nc.sync.drain()
            wait_clock.add_sem_waits(
                drain_inst.ins, ScopedClock({None: tick_clock.global_clock})
            )
            self.nc.all_engine_barrier(sem_only=True)
            popped = self.nc._tile_sem_poison_stack.pop()
            assert popped is self._sem_poison

    return SlimTailTileContext


def _build():
    import concourse.bacc as bacc
    import concourse.mybir as mybir

    TileContext = _make_tc_class()

    f32 = mybir.dt.float32
    bf16 = mybir.dt.bfloat16
    SQ = mybir.ActivationFunctionType.Square
    CP = mybir.ActivationFunctionType.Copy
    ALU = mybir.AluOpType
    HALF_SQRT = float(np.float32(np.sqrt(0.5)))

    i32 = mybir.dt.int32

    nc = bacc.Bacc()
    x = nc.declare_dram_parameter("inputs", [_BS, _ROW], f32, isOutput=False)
    out = nc.declare_dram_parameter("out", [_BS, _E], f32, isOutput=True)

    n_chunks = _BS // _P  # 2
    # DMA split per chunk: small first slice for an early compute start,
    # tiny last slice on the final chunk to shorten the critical tail
    halves_by_chunk = [
        [(0, 11), (11, 14)],
        [(0, 13), (13, 12)],
    ]
    groups_by_half = {11: [7, 4], 14: [7, 7], 13: [7, 6], 12: [7, 5]}

    with TileContext(nc) as tc:
        with (
            tc.tile_pool(name="consts", bufs=1) as cpool,
            tc.tile_pool(name="x", bufs=4) as xpool,
            tc.tile_pool(name="xb", bufs=4) as xbpool,
            tc.tile_pool(name="xT", bufs=4) as xtpool,
            tc.tile_pool(name="xsq", bufs=4) as sqpool,
            tc.tile_pool(name="pt", bufs=3, space="PSUM") as ptpool,
            tc.tile_pool(name="acc", bufs=2, space="PSUM") as accpool,
            tc.tile_pool(name="warmps", bufs=1, space="PSUM") as wpool,
            tc.tile_pool(name="small", bufs=2) as spool,
        ):
            # Warm op: forces the ACT function-table load off the critical
            # path (it otherwise lands right before the first Square, after
            # a cross-engine wait).
            warm = spool.tile([_P, 1], f32, tag="warm")
            nc.gpsimd.memset(warm[:], 0.0)
            nc.scalar.activation(warm[:], warm[:], SQ)

            # Constants built on-chip (a DMA for these queues behind the
            # input packets and stalls the first transposes by multiple us).
            # iota with channel_multiplier=-1 gives v[p,j] = j - p, so
            # identity = (v == 0); the stacked mask [128,64] has ones where
            # j - p is 0 or -64.
            iot_i = cpool.tile([_P, _P], i32, tag="iot_i")
            iot_m = cpool.tile([_P, _E], i32, tag="iot_m")
            ident = cpool.tile([_P, _P], bf16, tag="ident")
            mask = cpool.tile([_P, _E], bf16, tag="mask")
            mask_b = cpool.tile([_P, _E], bf16, tag="mask_b")
            nc.gpsimd.iota(iot_i[:], pattern=[[1, _P]], base=0, channel_multiplier=-1)
            nc.gpsimd.iota(iot_m[:], pattern=[[1, _E]], base=0, channel_multiplier=-1)
            nc.vector.tensor_scalar(
                ident[:], iot_i[:], 0, None, op0=ALU.is_equal
            )
            nc.vector.tensor_scalar(
                mask[:], iot_m[:], 0, None, op0=ALU.is_equal
            )
            nc.vector.tensor_scalar(
                mask_b[:], iot_m[:], -_E, None, op0=ALU.is_equal
            )
            nc.vector.tensor_add(mask[:], mask[:], mask_b[:])
            # Pre-scale the chain masks so the combine needs no scaling:
            # s-chain mask = sqrt(0.5)*one-hot -> s'^2 = 0.5*s^2 (up to the
            # bf16 rounding of sqrt(0.5): (c^2-0.5)*s^2 ~ 1e-4 rel, noise
            # next to the 2.4e-3 bf16-squares error); q-chain mask =
            # 0.5*one-hot (exact in bf16) -> q' = 0.5*q.
            maskh = cpool.tile([_P, _E], bf16, tag="maskh")
            maskq = cpool.tile([_P, _E], bf16, tag="maskq")
            nc.vector.tensor_scalar_mul(maskh[:], mask[:], HALF_SQRT)
            nc.vector.tensor_scalar_mul(maskq[:], mask[:], 0.5)

            # PE HAM warm-up: dummy matmuls under the first DMA's shadow
            # release the PE clock gate (1.2 -> 2.4 GHz) before real work.
            wmov = cpool.tile([_P, _P], bf16, tag="wmov")
            nc.gpsimd.memset(wmov[:], 0.0)
            wps = wpool.tile([_P, 512], f32, tag="wps")
            for _ in range(45):
                nc.tensor.matmul(wps[:, :_P], wmov[:], ident[:], start=True, stop=True)

            for c in range(n_chunks):
                rows = slice(c * _P, (c + 1) * _P)
                # separate banks: a start=True matmul clears its whole bank,
                # so the two accumulation chains must not share one
                s_t = accpool.tile([_P, 512], f32, tag="s")
                q_t = accpool.tile([_P, 512], f32, tag="q")
                s_ps = s_t[:, :_E]
                q_ps = q_t[:, :_E]
                for blk0, nblk in halves_by_chunk[c]:
                    cols = slice(blk0 * _P, (blk0 + nblk) * _P)
                    n = nblk * _P
                    xt = xpool.tile([_P, n], f32, tag="x")
                    nc.sync.dma_start(out=xt[:], in_=x[rows, cols])
                    xbt = xbpool.tile([_P, n], bf16, tag="xb")

                    xT = xtpool.tile([_P, n], bf16, tag="xT")
                    xsq = sqpool.tile([_P, n], bf16, tag="xsq")
                    g0 = 0
                    for gn in groups_by_half[nblk]:
                        gcols = slice(g0 * _P, (g0 + gn) * _P)
                        # per-group cast: lets the tail chunk pipeline at
                        # group granularity (gpsimd CAST measured 4x slower
                        # than DVE's 2x mode, so this stays on DVE)
                        nc.vector.tensor_copy(xbt[:, gcols], xt[:, gcols])
                        pt = ptpool.tile([_P, 7 * _P], bf16, tag="pt")
                        for j in range(gn):
                            k = g0 + j
                            nc.tensor.transpose(
                                pt[:, j * _P : (j + 1) * _P],
                                xbt[:, k * _P : (k + 1) * _P],
                                ident[:],
                            )
                        nc.vector.tensor_copy(xT[:, gcols], pt[:, : gn * _P])
                        nc.scalar.activation(xsq[:, gcols], pt[:, : gn * _P], SQ)
                        g0 += gn
                    for k in range(nblk):
                        kk = blk0 + k
                        bcols = slice(k * _P, (k + 1) * _P)
                        nc.tensor.matmul(
                            s_ps,
                            xT[:, bcols],
                            maskh[:],
                            start=(kk == 0),
                            stop=(kk == _NBLK - 1),
                        )
                        nc.tensor.matmul(
                            q_ps,
                            xsq[:, bcols],
                            maskq[:],
                            start=(kk == 0),
                            stop=(kk == _NBLK - 1),
                        )

                # res = s'^2 - q' = 0.5*s^2 - 0.5*q
                m2 = spool.tile([_P, _E], f32, tag="m2")
                res = spool.tile([_P, _E], f32, tag="res")
                nc.scalar.activation(m2[:], s_ps, SQ)
                nc.vector.tensor_sub(res[:], m2[:], q_ps)
                nc.sync.dma_start(out=out[rows, :], in_=res[:])
    nc.compile()
    return nc


_WALRUS_EXTRA = []


def _patch_walrus():
    """Cap walrus's semaphore allocation: the NEFF postamble zeroes every
    allocated semaphore one event-sem op at a time (spaced to dodge the
    event-accel erratum), so unused semaphores cost ~150ns each at the
    kernel tail."""
    from concourse import bass_utils

    if getattr(bass_utils, "_walrus_patched", False):
        return
    real_run = bass_utils.run_command

    def run2(cmd, **kw):
        if cmd and "walrus_driver" in str(cmd[0]):
            cmd = list(cmd) + _WALRUS_EXTRA
        return real_run(cmd, **kw)

    bass_utils.run_command = run2
    bass_utils._walrus_patched = True


def _run(in_maps, **kwargs):
    from concourse.bass_utils import run_bass_kernel_spmd

    _patch_walrus()
    nc = _build()
    return run_bass_kernel_spmd(nc, in_maps, core_ids=list(range(_NCORES)), **kwargs)


def _shard(inputs: np.ndarray):
    x = np.ascontiguousarray(
        np.asarray(inputs, dtype=np.float32).reshape(_B, _ROW)
    )
    return [
        {"inputs": np.ascontiguousarray(x[i * _BS : (i + 1) * _BS])}
        for i in range(_NCORES)
    ]


def kernel(
    inputs: np.ndarray,
    weight_attention: np.ndarray = None,
    weight_projection: np.ndarray = None,
    weight_bias: np.ndarray = None,
) -> np.ndarray:
    # weights are dead code (softmax over a size-1 axis == 1.0)
    res = _run(_shard(inputs))
    return np.concatenate([r["out"] for r in res.results], axis=0)

